# revision 58
# baseline (speedup 1.0000x reference)
"""MMD loss kernel for Trainium2 (8 NeuronCores, Bass/Tile).

reference math:
  src = X[:2048], tgt = X[2048:],  D=512
  xx = mean over [4096,4096] of sum_k exp(-d2_dup(src,src)/(bw_xx*2^k))
  (dup matrix mean == mean over the 2048^2 block), similarly yy, and
  xy uses the full 4096^2 matrix of X.
  bw for (a,b) = sum(d2([a;b]))/(m^2-m) / mul^(num//2),  mul=2, num=5.

Strategy (v2 — strip-major, fp8-DoubleRow, LDW-amortized):
  - bandwidth sums have a closed form: sum_block d2 = 2n*sum(sq) - 2|sum x|^2
    -> computed host-side in fp64, passed as runtime activation *scales*.
  - pairwise tile: PSUM M = G - sq_i/2 - sq_j/2 = -d2/2 via K=512 data in
    fp8-e4m3 DoubleRow (2 instructions of K=256) + a bf16 aug matmul with
    hi/lo split of -sq/2 (only the x.y cross term is fp8-rounded;
    full-pipeline rel err ~6.6e-4, budget 2e-2).
  - 5-kernel sum: u = exp(scale*M) on ACT (accum_out rider = sum u), then
    ONE fused custom-DVE op computes u^2+u^4+u^8+u^16 (accum_out rider =
    its row sum). Host only needs the total of the 5 kernel sums.
  - coverage/symmetry: per half, each core computes its diagonal block
    (S00+S11 at w1 + S01 at w2), one adjacent pair block (w2), and two
    diagonal QUARTERS of its skip pair using ITS OWN rows as lhsT (the
    partner core covers the anti-diagonal quarters; which partner rows
    land in the skipA/skipB sections is per-core data placement, so the
    program stays SPMD). Cross src/tgt blocks are covered once with
    weight 2 across the 8 cores.
  - strip-major tiles: tile T_s gathers ALL rhs columns contracted
    against own-row strip s (adj | diag2 | skip | c0 | c1), so the whole
    tile needs just 3 stationary weights (aug_s, fp8 pair01_s, pair23_s).
    A post-scheduling pass deletes back-to-back duplicate LDWEIGHTS
    (verified on HW: matmuls reuse the loaded array), amortizing the
    DoubleRow weight-load (256 cols, ~213ns, not prefetchable) over
    ~1.5-2.5us of streaming per tile. T4 holds diag1 (S00+S11, w1).
  - chain regions are weight-class-uniform: T_s own-chain = [adj|diag2|
    skip] prefix (w2), xy-chain = whole tile (w2); T4 both chains w1.
  - timing contract: only the final rep's riders are DMA'd out, so the
    output size is independent of REPEAT and the wall-delta between
    REPEAT variants isolates on-device body time.
"""

import sys

sys.path.insert(0, "/opt/trn_rl_repo")

import numpy as np
import ml_dtypes

N, D, HALF = 4096, 512, 2048
NCORES = 8
RID_W = 2           # rider slots per unit: [sum u, sum u^2+u^4+u^8+u^16]

# Local column layout (units of X-row indices), per core:
#   own   [0:512)      core's own row group g = 4*half + k0
#   adj   [512:1024)   group (k0+1)%4 of the same half      -> pair w2
#   skipA [1024:1280)  skip-pair partner group (k0+2)%4 rows C1
#   skipB [1280:1536)  partner rows C2
#     k0<2: C1=partner[0:256],  C2=partner[256:512]  (diagonal quarters)
#     k0>=2: C1=partner[256:512], C2=partner[0:256] (anti-diagonal) — the
#     two cores of a pair tile the block exactly, both w2, own-row lhsT.
#   cross [1536:2560)  two 512-col groups of the other half -> w2, xy only
LC = 2560
AUG_W = LC + 512    # + [1,1,hi,lo] lhsT section for own rows
OWN_OFF, ADJ_OFF, SKA_OFF, SKB_OFF = 0, 512, 1024, 1280
CR0_OFF, CR1_OFF = 1536, 2048

# Tiles (all PSUM [128,1536] = 3 banks, double-buffered; 1 spare bank
# holds the Σu colsum accumulators):
#   U_s (s=0..3): strip-major over own strip s. U0/U1 carry
#     [adj|diag2|skip|c0]; U2/U3 carry [adj|c0|c1]. U5 = [c1_s0|c1_s1].
#     U4 = diag1 (w1) + skip strips 2,3 (w2).
#   regions: (po, w, rhs_off, bank_start); own-chain = [0:own_end),
#   xy = [0:used). PSUM start=True pending-zeroes a whole 2KB bank, so
#   exactly ONE region per bank carries bank_start=True (the others
#   accumulate onto the zeroed bytes); all MM outputs stay within a bank.
def _tile_plan(s):
    if s < 2:
        regions = [(0, 512, ADJ_OFF, True), (512, 256, OWN_OFF + 256, True),
                   (768, 256, SKA_OFF, False), (1024, 512, CR0_OFF, True)]
        return regions, 1024, 1536
    regions = [(0, 512, ADJ_OFF, True), (512, 512, CR0_OFF, True),
               (1024, 512, CR1_OFF, True)]
    return regions, 512, 1536


# U4: diag1 (S00+S11, w1) at [0:1024) + skip strips 2,3 (w2) at
# [1024:1536). (po, w, rhs_off, strip, bank_start)
# bank0's start flag sits on d1_s1 because U4's strip order is (1,0,2,3)
T4_REGIONS = [(0, 256, OWN_OFF, 0, False),
              (256, 256, OWN_OFF, 1, True),
              (512, 256, OWN_OFF + 256, 2, True),
              (768, 256, OWN_OFF + 256, 3, False),
              (1024, 256, SKB_OFF, 2, True),
              (1280, 256, SKB_OFF, 3, False)]

# U5: c1 for strips 0,1. (po, w, rhs_off, strip, bank_start)
T5_REGIONS = [(0, 512, CR1_OFF, 0, True),
              (512, 512, CR1_OFF, 1, True)]

# units (DVE powers rider per unit; Σu from ACT riders for xy chains
# u4-u7/u12, from PE colsum slots for the rest):
# 0-3 U_s own (w2), 4-7 U_s xy (w2), 8 diag1 own (w1), 9 diag1 xy (w1),
# 10 skip23 own (w2), 11 skip23 xy (w2), 12 U5 xy (w2)
NUNIT = 13
# colsum accumulator slots (partition of the acc bank): Σu classes
SLOT_OWN_W2, SLOT_OWN_W1, SLOT_XY_W1, SLOT_XY_W2 = 0, 32, 64, 96

USE_DR = True       # fp8-e4m3 DoubleRow data matmuls (bf16 if False)
U_DT = "float32"    # dtype of the exp output / fused-op scratch tiles

REPEAT = 1

_MMD_OP = None


def _get_mmd_op():
    """Fused DVE op: out = u^2+u^4+u^8+u^16, accum_out = row-sum(out)."""
    global _MMD_OP
    if _MMD_OP is not None:
        return _MMD_OP
    from concourse import dve_ops
    from concourse.dve_spec import Spec, Src0, sq, lower
    from concourse.dve_uop import AluOp, DveOpSpec

    name = "MMD_POW_SUM"
    for op in dve_ops.OPS:
        if op.name == name:
            _MMD_OP = op
            return op

    a = sq(Src0)
    b = sq(a)
    c = sq(b)
    d = sq(c)

    def _ref(in0, in1, c0, c1, c2):
        x = in0.astype(np.float32)
        aa = x * x
        bb = aa * aa
        cc = bb * bb
        dd = cc * cc
        body = (aa + bb) + (cc + dd)
        return body, body.reshape(body.shape[0], -1).sum(
            axis=-1, keepdims=True)

    spec = Spec(body=(a + b) + (c + d), accum=AluOp.ADD, reference=_ref)
    row = max(dve_ops._SUB_OPCODE_FOR_NAME.values()) + 1
    assert row < 0x20, "custom-DVE opcode rows exhausted"
    shas = {}
    for ver in ("v3", "v4"):
        uops = lower(spec, ver=ver)
        shas[ver] = DveOpSpec(name=name, opcode=row, uops=uops,
                              rd1_en=False).sha(ver)
    op = dve_ops.DveOp(name, spec, subdim=False, uops_sha=shas)
    dve_ops.OPS.append(op)
    dve_ops._SUB_OPCODE_FOR_NAME[name] = row
    dve_ops.CUSTOM_DVE_SPECS[name] = spec
    _MMD_OP = op
    return op


def _local_cols(core):
    half, k = core // 4, core % 4
    own_base, other_base = half * HALF, (1 - half) * HALF
    own = own_base + 512 * k + np.arange(512)
    adj = own_base + 512 * ((k + 1) % 4) + np.arange(512)
    pbase = own_base + 512 * ((k + 2) % 4)
    if k < 2:
        ska = pbase + np.arange(256)
        skb = pbase + 256 + np.arange(256)
    else:
        ska = pbase + 256 + np.arange(256)
        skb = pbase + np.arange(256)
    if half == 0:
        cross = [0, 1] if k % 2 == 0 else [2, 3]
    else:
        cross = [1, 3] if k < 2 else [0, 2]
    cr = [other_base + 512 * b + np.arange(512) for b in cross]
    return np.concatenate([own, adj, ska, skb] + cr)


def _dedup_ldweights(nc):
    """Remove back-to-back InstLdweights with identical weights AP.

    HW-verified: matmuls keep using the currently-loaded stationary
    operand, so only the first load of a run is needed. Conservative:
    a duplicate is dropped only if its dependency sets are subsets of
    the kept load's.
    """
    removed = kept = 0
    for b in nc.m.functions[0].blocks:
        insts = b.instructions
        last_key = None
        last_ld = None
        out = []
        changed = False
        for i in insts:
            if type(i).__name__ == "InstLdweights":
                key = (str(i.ins[0]), str(i.perf_mode),
                       str(getattr(i, "tile_position", None)),
                       str(getattr(i, "tile_size", None)))
                if (last_key == key
                        and set(i.sync_dependency_names())
                        <= set(last_ld.sync_dependency_names())
                        and set(i.nosync_dependency_names())
                        <= set(last_ld.nosync_dependency_names())):
                    removed += 1
                    changed = True
                    continue
                last_key, last_ld = key, i
                kept += 1
            out.append(i)
        if changed:
            b.instructions = out
    return removed, kept


def _build_program():
    import concourse.bacc as bacc
    import concourse.mybir as mybir
    import concourse.tile as tile

    f32 = mybir.dt.float32
    bf16 = mybir.dt.bfloat16
    dr_dt = mybir.dt.float8e4
    u_dt = getattr(mybir.dt, U_DT)
    mmd_op = _get_mmd_op()
    DR = mybir.MatmulPerfMode.DoubleRow

    nc = bacc.Bacc("TRN2", target_bir_lowering=False, debug=False,
                   num_devices=NCORES)
    if USE_DR:
        xth_d = nc.dram_tensor("xth", [128, 4, LC], dr_dt,
                               kind="ExternalInput")
    else:
        xth_d = nc.dram_tensor("xth", [4, 128, LC], bf16,
                               kind="ExternalInput")
    aug_d = nc.dram_tensor("aug", [4, AUG_W], bf16, kind="ExternalInput")
    sc_d = nc.dram_tensor("scales", [128, 2], f32, kind="ExternalInput")
    ones_d = nc.dram_tensor("ones", [128, 1], mybir.dt.float16,
                            kind="ExternalInput")
    nrep = globals().get("REPEAT", 1)
    rid_d = nc.dram_tensor("riders", [NUNIT, 128, RID_W], f32,
                           kind="ExternalOutput")
    usum_d = nc.dram_tensor("usums", [4, 512], f32, kind="ExternalOutput")

    with tile.TileContext(nc) as tc:
        with (
            tc.tile_pool(name="xtp", bufs=1) as xtp,
            tc.tile_pool(name="augp", bufs=1) as augp,
            tc.tile_pool(name="scp", bufs=1) as scp,
            tc.tile_pool(name="ridp", bufs=1) as ridp,
            tc.tile_pool(name="psp", bufs=8, space="PSUM") as psp,
            tc.tile_pool(name="up", bufs=4) as up,
        ):
            if USE_DR:
                xq = xtp.tile([128, 4, LC], dr_dt, tag="xq", name="xq")
                nc.sync.dma_start(out=xq[:], in_=xth_d.ap())
                xth = None
            else:
                xth = [xtp.tile([128, LC], bf16, tag=f"xth{k}",
                                name=f"xth{k}") for k in range(4)]
                for k in range(4):
                    nc.sync.dma_start(out=xth[k][:], in_=xth_d.ap()[k])
            aug = augp.tile([4, AUG_W], bf16, tag="aug", name="aug")
            sc = scp.tile([128, 2], f32, tag="sc", name="sc")
            ones = scp.tile([128, 1], mybir.dt.float16, tag="ones",
                            name="ones")
            nc.sync.dma_start(out=aug[:], in_=aug_d.ap())
            nc.sync.dma_start(out=sc[:], in_=sc_d.ap())
            nc.sync.dma_start(out=ones[:], in_=ones_d.ap())

            riders = [[ridp.tile([128, RID_W], f32, tag=f"rid{u}_{rp}",
                                 name=f"rid{u}_{rp}") for u in range(NUNIT)]
                      for rp in range(nrep)]
            # Σu colsum accumulators: one PSUM bank, 4 class slots at
            # partitions 0/32/64/96, alive across the whole kernel.
            # Allocated lazily so colsum-free builds don't hold the bank.
            _acc_box = {}

            def get_acc():
                if "acc" not in _acc_box:
                    _acc_box["acc"] = psp.tile([128, 512], f32, tag="acc",
                                               name="acc", bufs=1)
                return _acc_box["acc"]

            ndata = 2 if USE_DR else 4

            def aug_mm(pss, s, off, w, bank_start, stop=False):
                nc.tensor.matmul(
                    out=pss,
                    lhsT=aug[:, LC + 128 * s:LC + 128 * s + 128],
                    rhs=aug[:, off:off + w],
                    start=bank_start, stop=stop)

            def data_mm(pss, s, off, w, p2, last, start=False):
                if USE_DR:
                    lh = xq[:, 2 * p2:2 * p2 + 2,
                            OWN_OFF + 128 * s:OWN_OFF + 128 * s + 128]
                    rh = xq[:, 2 * p2:2 * p2 + 2, off:off + w]
                    nc.tensor.matmul(out=pss, lhsT=lh, rhs=rh,
                                     start=start, stop=last, perf_mode=DR)
                else:
                    lh = xth[p2][:, OWN_OFF + 128 * s:
                                 OWN_OFF + 128 * s + 128]
                    rh = xth[p2][:, off:off + w]
                    nc.tensor.matmul(out=pss, lhsT=lh, rhs=rh,
                                     start=False, stop=last)

            colsum_state = {}

            def colsum(cur, l, h, slot):
                """Σ over partitions+cols of cur[:, l:h) accumulated into
                acc[slot, 0:512) via ones-lhsT matmuls. PSUM start=True
                pending-zeroes 2KB in the OUTPUT's partitions only, so
                each slot's first colsum per rep carries its own
                start=True; later ones accumulate."""
                acc = get_acc()
                for c in range(l, h, 512):
                    w = min(512, h - c)
                    nc.tensor.matmul(
                        out=acc[slot:slot + 1, 0:w],
                        lhsT=ones[:, 0:1], rhs=cur[:, c:c + w],
                        start=colsum_state.get(slot, True), stop=False,
                        tile_position=(0, slot))
                    colsum_state[slot] = False

            def chain(ps, rep, lo, hi, sci, splits):
                """ACT exp over ps[lo:hi) then per-split DVE powers.

                splits: list of (unit, l, h, slot) sub-ranges of
                [lo, hi) at rider-class granularity. slot=None: Σu via
                the ACT accumulator rider (requires a single split);
                else Σu via PE colsums into the acc-bank class slot.
                """
                cur = up.tile([128, 1536], mybir.dt.float16, tag="u",
                              name="u", bufs=3)
                act_kw = {}
                if splits[0][3] is None:
                    (unit, _, _, _), = splits
                    act_kw["accum_out"] = riders[rep][unit][:, 0:1]
                nc.scalar.activation(
                    out=cur[:, 0:hi - lo], in_=ps[:, lo:hi],
                    func=mybir.ActivationFunctionType.Exp,
                    scale=sc[:, sci:sci + 1], **act_kw)
                scr = up.tile([128, 1536], mybir.dt.float16, tag="usq",
                              name="usq", bufs=2)
                for unit, l, h, slot in splits:
                    if slot is not None:
                        colsum(cur, l - lo, h - lo, slot)
                    nc.vector._custom_dve(
                        mmd_op, out=scr[:, l - lo:h - lo],
                        in0=cur[:, l - lo:h - lo],
                        accum_out=riders[rep][unit][:, 1:2])

            skip_chains = globals().get("SKIP_CHAINS", False)
            if skip_chains:
                def chain(*a, **kw):  # noqa: F811
                    return None

            def weight_major_rev(ps, region_list, strip_order):
                # (p23, p01, aug) per strip: p23 carries the bank-start
                # flags, aug carries stop.
                for s in strip_order:
                    regs = [r for r in region_list if r[3] == s]
                    for p2 in reversed(range(ndata)):
                        for po, w, off, _, bst in regs:
                            data_mm(ps[:, po:po + w], s, off, w, p2,
                                    last=False,
                                    start=(bst and p2 == ndata - 1))
                    for po, w, off, _, bst in regs:
                        aug_mm(ps[:, po:po + w], s, off, w, False,
                               stop=True)

            def weight_major(ps, region_list):
                # aug group first (carries the bank start flags), then
                # the data weight groups; regions grouped by strip; the
                # LDW-dedup pass collapses each group to one load.
                strips = sorted({r[3] for r in region_list})
                for s in strips:
                    regs = [r for r in region_list if r[3] == s]
                    for po, w, off, _, bst in regs:
                        aug_mm(ps[:, po:po + w], s, off, w, bst)
                    for p2 in range(ndata):
                        for po, w, off, _, bst in regs:
                            data_mm(ps[:, po:po + w], s, off, w, p2,
                                    last=(p2 == ndata - 1))

            # COLSUMS: which Σu riders go through PE colsums (vs ACT
            # accumulators). 0 = ACT riders everywhere (U4 chains split
            # per class), 1 = U4 only, 2 = U4 + U_s own.
            colsum_level = globals().get("COLSUM_LEVEL", 2)
            globals()["_BUILT_COLSUM_LEVEL"] = colsum_level

            for rep in range(nrep):
                colsum_state.clear()
                for s in range(4):
                    regions, own_end, used = _tile_plan(s)
                    ps = psp.tile([128, 1536], f32, tag="ps", name="ps",
                                  bufs=2)
                    weight_major(ps, [(po, w, off, s, bst)
                                      for po, w, off, bst in regions])
                    own_slot = SLOT_OWN_W2 if colsum_level >= 2 else None
                    xy_slot = SLOT_XY_W2 if colsum_level >= 3 else None
                    chain(ps, rep, 0, own_end, 0,
                          [(s, 0, own_end, own_slot)])
                    chain(ps, rep, 0, used, 1, [(4 + s, 0, used, xy_slot)])

                # U5: c1 columns for strips 0,1 (xy-only, w2)
                ps = psp.tile([128, 1536], f32, tag="ps", name="ps",
                              bufs=2)
                weight_major(ps, T5_REGIONS)
                chain(ps, rep, 0, 1024, 1,
                      [(12, 0, 1024,
                        SLOT_XY_W2 if colsum_level >= 3 else None)])

                # U4: diag1 (w1) + skip strips 2,3 (w2); merged ACT per
                # set, Σu split via colsum class slots.
                ps = psp.tile([128, 1536], f32, tag="ps", name="ps",
                              bufs=2)
                weight_major_rev(ps, T4_REGIONS, (1, 0, 2, 3))
                if colsum_level >= 1:
                    chain(ps, rep, 0, 1536, 0,
                          [(8, 0, 1024, SLOT_OWN_W1),
                           (10, 1024, 1536, SLOT_OWN_W2)])
                    chain(ps, rep, 0, 1536, 1,
                          [(9, 0, 1024, SLOT_XY_W1),
                           (11, 1024, 1536, SLOT_XY_W2)])
                else:
                    chain(ps, rep, 0, 1024, 0, [(8, 0, 1024, None)])
                    chain(ps, rep, 1024, 1536, 0, [(10, 1024, 1536, None)])
                    chain(ps, rep, 0, 1024, 1, [(9, 0, 1024, None)])
                    chain(ps, rep, 1024, 1536, 1, [(11, 1024, 1536, None)])

            if skip_chains:
                for u in range(NUNIT):
                    nc.gpsimd.memset(riders[nrep - 1][u][:], 0.0)
            for u in range(NUNIT):
                nc.sync.dma_start(out=rid_d.ap()[u],
                                  in_=riders[nrep - 1][u][:])
            accs = scp.tile([128, 512], f32, tag="accs", name="accs")
            if colsum_level >= 1 and not skip_chains:
                nc.vector.tensor_copy(accs[:], get_acc()[:])
            else:
                nc.vector.memset(accs[:], 0.0)
            for i, slot in enumerate((SLOT_OWN_W2, SLOT_OWN_W1,
                                      SLOT_XY_W1, SLOT_XY_W2)):
                nc.sync.dma_start(out=usum_d.ap()[i],
                                  in_=accs[slot:slot + 1, 0:512])

    nrem, nkept = _dedup_ldweights(nc)
    nc.compile()
    return nc


_PROG = None


def _get_program():
    global _PROG
    if _PROG is None:
        _PROG = _build_program()
    return _PROG


def _prep_inputs(latent):
    X = np.asarray(latent, np.float32)
    X64 = X.astype(np.float64)
    sq = (X64 * X64).sum(1)                      # [N]
    M2 = float(N) * N - N

    def block_d2_sum(lo, hi):
        n = hi - lo
        sv = X64[lo:hi].sum(0)
        return 2.0 * (n * sq[lo:hi].sum()) - 2.0 * (sv @ sv)

    S_src = block_d2_sum(0, HALF)
    S_tgt = block_d2_sum(HALF, N)
    sv_all = X64.sum(0)
    S_full = 2.0 * (N * sq.sum()) - 2.0 * (sv_all @ sv_all)

    bw_xx = S_src / M2           # already includes /mul^(num//2)
    bw_yy = S_tgt / M2
    bw_xy = (S_full / M2) / 4.0

    in_maps = []
    for core in range(NCORES):
        lc = _local_cols(core)
        xf = X[lc].T.reshape(4, 128, LC)
        if USE_DR:
            xth = np.ascontiguousarray(xf.transpose(1, 0, 2)).astype(
                ml_dtypes.float8_e4m3)
        else:
            xth = np.ascontiguousarray(xf).astype(ml_dtypes.bfloat16)
        sql = sq[lc]
        v = -0.5 * sql
        hi = np.asarray(v, ml_dtypes.bfloat16).astype(np.float64)
        lo = (v - hi).astype(np.float32)
        hi = hi.astype(np.float32)
        ones = np.ones_like(hi)
        augm = np.zeros((4, AUG_W), ml_dtypes.bfloat16)
        augm[0, :LC] = hi
        augm[1, :LC] = lo
        augm[2, :LC] = ones
        augm[3, :LC] = ones
        # lhsT section: [1, 1, hi_row, lo_row] for own rows
        augm[0, LC:LC + 512] = 1.0
        augm[1, LC:LC + 512] = 1.0
        augm[2, LC:LC + 512] = hi[OWN_OFF:OWN_OFF + 512]
        augm[3, LC:LC + 512] = lo[OWN_OFF:OWN_OFF + 512]

        bw_own = bw_xx if core < 4 else bw_yy
        scales = np.zeros((128, 2), np.float32)
        scales[:, 0] = 1.0 / (8.0 * bw_own)
        scales[:, 1] = 1.0 / (8.0 * bw_xy)
        in_maps.append({"xth": xth, "aug": augm, "scales": scales,
                        "ones": np.ones((128, 1), np.float16)})
    return in_maps


def _postprocess(results):
    level = globals().get("_BUILT_COLSUM_LEVEL", 2)
    S_own = np.zeros(NCORES)
    S_xy = np.zeros(NCORES)
    for core in range(NCORES):
        r = results[core]["riders"].astype(np.float64)  # [NUNIT,128,RID_W]
        us = results[core]["usums"].astype(np.float64).sum(1)  # [4]
        pw = r[:, :, 1].sum(1)                 # Σpowers per unit
        au = r[:, :, 0].sum(1)                 # ACT Σu riders

        def sig(u, colsummed):
            return pw[u] + (0.0 if colsummed else au[u])

        cs_own = level >= 2     # units 0-3 colsummed?
        cs_t4 = level >= 1      # units 8-11 colsummed?
        cs_xy = level >= 3      # units 4-7, 12 colsummed?
        # Σu colsum classes: 0=own_w2, 1=own_w1, 2=xy_w1, 3=xy_w2
        S_own[core] = 2.0 * (sig(0, cs_own) + sig(1, cs_own)
                             + sig(2, cs_own) + sig(3, cs_own)
                             + sig(10, cs_t4)) + sig(8, cs_t4)
        S_xy[core] = 2.0 * (sig(4, cs_xy) + sig(5, cs_xy) + sig(6, cs_xy)
                            + sig(7, cs_xy) + sig(12, cs_xy)
                            + sig(11, cs_t4)) + sig(9, cs_t4)
        if level >= 1:
            S_own[core] += 2.0 * us[0] + us[1]
            S_xy[core] += 2.0 * us[3] + us[2]
    xx = S_own[:4].sum() / (HALF * HALF)
    yy = S_own[4:].sum() / (HALF * HALF)
    xy = S_xy.sum() / (float(N) * N)
    return np.float32(xx + yy - 2.0 * xy)


def _run(inputs, trace=False, **kw):
    from concourse.bass_utils import run_bass_kernel_spmd
    nc = _get_program()
    in_maps = _prep_inputs(inputs["latent"])
    res = run_bass_kernel_spmd(nc, in_maps, list(range(NCORES)),
                               trace=trace, **kw)
    return _postprocess(res.results), res


def kernel(**inputs):
    out, _ = _run(inputs, trace=False)
    return out


if __name__ == "__main__":
    rng = np.random.default_rng(0)
    lat = rng.standard_normal((N, D)).astype(np.float32)
    print(kernel(latent=lat,
                 domain=np.concatenate([np.zeros(HALF, np.int32),
                                        np.ones(HALF, np.int32)])))
